# revision 2
# baseline (speedup 1.0000x reference)
"""ContrastiveDist kernel for TRN2 (8 NeuronCores, SPMD).

out[n] = sum_e -(t_e . v_n) / (||t_e|| * ||v_n|| + eps)
       = -(s . v_n) / (||v_n|| + eps')        with s = sum_e t_e / ||t_e||
(the eps placement differs from the reference by O(eps/||.||^2) ~ 1e-10
relative -- far below fp32 noise).

Sharding: node_emb split across 8 cores on the node axis (6250 rows each,
padded to 6272 = 49*128); target replicated. Each core:
  phase A: ssq_e -> 1/||t_e|| -> t_hat (DVE) -> column-sum via PE matmul
           with a ones stationary vector -> s [1,256] -> broadcast to
           s_b [128,256] (GpSimd partition_broadcast).
  phase B: for each of 49 node tiles [128,256] (node n = p*49 + t so the
           final [128,49] result DMAs out partition-contiguous):
           fused mul+reduce (DVE tensor_tensor_reduce) for -v.s, and
           square+accum (ACT activation / DVE ttr, alternating) for ssq_v.
  tail:    sqrt, +eps, reciprocal, multiply, single 25KB DMA out.
"""

import numpy as np
from contextlib import ExitStack

import concourse.bacc as bacc
import concourse.bass as bass
import concourse.mybir as mybir
import concourse.tile as tile
from concourse import bass_utils

EPS = 1e-8
E, D = 2048, 256          # entities, embed dim
N_FULL = 50000            # total nodes
N_CORES = 8
NPC = N_FULL // N_CORES   # 6250 true nodes per core
TPC = 49                  # node tiles per core (49*128 = 6272 padded)
NPAD = TPC * 128
ET = E // 128             # 16 entity tiles

F32 = mybir.dt.float32

_cache = {}


def _build():
    nc = bacc.Bacc(
        "TRN2",
        target_bir_lowering=False,
        debug=False,
        enable_asserts=True,
        num_devices=N_CORES,
    )
    tgt = nc.dram_tensor("target", [E, D], F32, kind="ExternalInput").ap()
    nodes = nc.dram_tensor("nodes", [NPAD, D], F32, kind="ExternalInput").ap()
    out = nc.dram_tensor("out", [NPAD], F32, kind="ExternalOutput").ap()

    with tile.TileContext(nc) as tc, ExitStack() as ctx:
        # all 16 target tiles stay resident (2MB) -- the normalize pass
        # needs 1/||t|| which depends on every tile's ssq
        tpool = ctx.enter_context(tc.tile_pool(name="tgt", bufs=ET))
        hpool = ctx.enter_context(tc.tile_pool(name="that", bufs=4))
        vpool = ctx.enter_context(tc.tile_pool(name="v", bufs=12))
        spool = ctx.enter_context(tc.tile_pool(name="small", bufs=1))
        scr_pool = ctx.enter_context(tc.tile_pool(name="scr", bufs=2))
        scr2_pool = ctx.enter_context(tc.tile_pool(name="scr2", bufs=2))
        psum = ctx.enter_context(tc.tile_pool(name="psum", bufs=1, space="PSUM"))

        ones_col = spool.tile([128, 1], F32)
        nc.vector.memset(ones_col[:], 1.0)

        # ---- phase A: s = sum_e target[e] / ||target[e]|| ----
        ssq_t = spool.tile([128, ET], F32)
        tgt_t = tgt.rearrange("(i p) d -> i p d", p=128)
        t_tiles = []
        for i in range(ET):
            t_i = tpool.tile([128, D], F32)
            nc.sync.dma_start(t_i[:], tgt_t[i])
            scr = scr_pool.tile([128, D], F32)
            nc.scalar.activation(
                scr[:], t_i[:], mybir.ActivationFunctionType.Square,
                accum_out=ssq_t[:, i : i + 1],
            )
            t_tiles.append(t_i)

        tn = spool.tile([128, ET], F32)
        nc.scalar.sqrt(tn[:], ssq_t[:])
        inv_tn = spool.tile([128, ET], F32)
        nc.vector.reciprocal(inv_tn[:], tn[:])

        ps = psum.tile([1, D], F32)
        for i in range(ET):
            that = hpool.tile([128, D], F32)
            nc.vector.tensor_scalar_mul(that[:], t_tiles[i][:], inv_tn[:, i : i + 1])
            nc.tensor.matmul(
                ps[:], ones_col[:], that[:], start=(i == 0), stop=(i == ET - 1)
            )

        s_row = spool.tile([1, D], F32)
        nc.scalar.copy(s_row[:], ps[:])
        s_b = spool.tile([128, D], F32)
        nc.gpsimd.partition_broadcast(s_b[:], s_row[:])

        # ---- phase B: per node tile, -v.s and ssq_v ----
        negdot = spool.tile([128, TPC], F32)
        ssq_v = spool.tile([128, TPC], F32)
        # node n = p*TPC + t: tile t holds partition-strided rows so the
        # result tile [128, TPC] is partition-contiguous in DRAM
        nodes_t = nodes.rearrange("(p t) d -> t p d", t=TPC)
        for t in range(TPC):
            v = vpool.tile([128, D], F32)
            nc.sync.dma_start(v[:], nodes_t[t])
            # tensor_tensor_reduce would fuse these but crashes the
            # bass2jax/PJRT execute path -- use mul + reduce instead
            scr1 = scr_pool.tile([128, D], F32, tag="scrB")
            nc.vector.tensor_mul(scr1[:], v[:], s_b[:])
            nc.vector.tensor_reduce(
                negdot[:, t : t + 1], scr1[:],
                axis=mybir.AxisListType.X, op=mybir.AluOpType.add, negate=True,
            )
            scr2 = scr2_pool.tile([128, D], F32)
            nc.scalar.activation(
                scr2[:], v[:], mybir.ActivationFunctionType.Square,
                accum_out=ssq_v[:, t : t + 1],
            )

        vn = spool.tile([128, TPC], F32)
        nc.scalar.sqrt(vn[:], ssq_v[:])
        vne = spool.tile([128, TPC], F32)
        nc.vector.tensor_scalar_add(vne[:], vn[:], EPS)
        inv_vn = spool.tile([128, TPC], F32)
        nc.vector.reciprocal(inv_vn[:], vne[:])
        res = spool.tile([128, TPC], F32)
        nc.vector.tensor_mul(res[:], negdot[:], inv_vn[:])
        nc.sync.dma_start(out.rearrange("(p t) -> p t", t=TPC), res[:])

    nc.compile()
    return nc


def _get_nc():
    if "nc" not in _cache:
        _cache["nc"] = _build()
    return _cache["nc"]


def run(pred, target, node_emb, trace=False, **trace_kwargs):
    """Returns (full_output [50000] f32, BassKernelResults)."""
    target = np.ascontiguousarray(np.asarray(target, dtype=np.float32))
    node_emb = np.ascontiguousarray(np.asarray(node_emb, dtype=np.float32))

    nc = _get_nc()
    in_maps = []
    for c in range(N_CORES):
        shard = np.empty((NPAD, D), dtype=np.float32)
        shard[:NPC] = node_emb[c * NPC : (c + 1) * NPC]
        shard[NPC:] = node_emb[: NPAD - NPC]  # pad with real rows (no 0-norm)
        in_maps.append({"target": target, "nodes": shard})

    res = bass_utils.run_bass_kernel_spmd(
        nc, in_maps, list(range(N_CORES)), trace=trace, **trace_kwargs
    )
    parts = [res.results[c]["out"][:NPC] for c in range(N_CORES)]
    return np.concatenate(parts).astype(np.float32), res


def kernel(pred, target, node_emb):
    out, _ = run(pred, target, node_emb)
    return out
